# revision 1
# baseline (speedup 1.0000x reference)
"""Trainium2 Bass kernel for nn_Attention_79224966742132.

Dense transformer attention block: QKV projection + axial RoPE + SDPA +
output projection, for x (2, 2048, 1152), 16 heads of dim 72.

Sharding (8 cores): data-parallel over batch (2) x tensor-parallel over
head groups (4 heads/core). Each core computes QKV for its 4 heads from
the full x[b], applies RoPE, runs attention, and produces a partial
output projection (row-parallel Wproj); the host sums the 4 partials per
batch element. The projection bias rides on the g==0 core of each batch.

All matmuls run in float32r (8-bit exp / 11-bit mantissa, 1 cycle/row on
the PE at moving dim >= 256 -- 4x faster than fp32 with ~1.2e-4 input
rounding). Scores are computed transposed (k tokens on partitions) so the
attention-weights @ V matmul needs no transposes; the softmax denominator
comes for free from an all-ones column appended to V. No max subtraction
is needed: scores*scale stay in a few units for this distribution.
"""
import math
import os
import sys

# The device path needs the axon/neuron jax platform; if a harness pinned
# JAX_PLATFORMS=cpu (common for running jax references) and jax is not yet
# imported, restore platform auto-detection.
if "jax" not in sys.modules:
    _jp = os.environ.get("JAX_PLATFORMS")
    if _jp and "axon" not in _jp and "neuron" not in _jp:
        del os.environ["JAX_PLATFORMS"]

import numpy as np

import bass_rust
import concourse.bass as bass
import concourse.mybir as mybir
import concourse.tile as tile
from concourse.bass_utils import run_bass_kernel_spmd

F32 = mybir.dt.float32
F32R = mybir.dt.float32r
AF = mybir.ActivationFunctionType
ALU = mybir.AluOpType

B = 2
N = 2048          # tokens = T*H*W = 8*16*16
C = 1152
NH = 16
HD = 72
ROT = 48          # rotary dims per head (24 pairs)
HPG = 4           # heads per core (16 heads / 4 groups)
NCORES = 8
GT, GH, GW = 8, 16, 16
SCALE = 1.0 / math.sqrt(HD)

NQ = 4            # token quarters in phase 1 / q-chunks in phase 2
QS = N // NQ      # 512
KTILES = N // 128  # 16 k-tiles
CK = C // 128     # 9 contraction chunks


def round_f32r(x: np.ndarray) -> np.ndarray:
    """Round fp32 to the float32r grid (RNE to 11 mantissa bits)."""
    bits = np.ascontiguousarray(x, dtype=np.float32).view(np.uint32)
    low = bits & np.uint32(0xFFF)
    hi = bits & np.uint32(0xFFFFF000)
    up = (low > 0x800) | ((low == 0x800) & (((bits >> 12) & 1) == 1))
    return (hi + np.where(up, np.uint32(0x1000), np.uint32(0))).view(np.float32)


def _axis_freqs(n: int) -> np.ndarray:
    base = np.linspace(1.0, 128.0, 8, dtype=np.float64) * np.pi   # MAX_FREQ/2 = 128
    pos = np.linspace(-1.0, 1.0, n, dtype=np.float64)
    return pos[:, None] * base[None, :]                            # (n, 8)


def _cos_sin_96():
    """cos/sin of the 24 pair frequencies per token, tiled x4 heads -> (96, N)."""
    f = np.zeros((GT, GH, GW, 24), dtype=np.float64)
    f[..., 0:8] = _axis_freqs(GT)[:, None, None, :]
    f[..., 8:16] = _axis_freqs(GH)[None, :, None, :]
    f[..., 16:24] = _axis_freqs(GW)[None, None, :, :]
    f = f.reshape(N, 24)
    cos24 = np.ascontiguousarray(np.cos(f).astype(np.float32).T)   # (24, N)
    sin24 = np.ascontiguousarray(np.sin(f).astype(np.float32).T)
    return np.tile(cos24, (4, 1)), np.tile(sin24, (4, 1))          # (96, N)


def build_nc() -> bass.Bass:
    nc = bass.Bass()
    xT = nc.dram_tensor("xT", [C, N], F32R, kind="ExternalInput")
    wqk = nc.dram_tensor("wqk", [C, 6 * 96], F32R, kind="ExternalInput")
    wv = nc.dram_tensor("wv", [C, HPG * HD], F32R, kind="ExternalInput")
    wp = nc.dram_tensor("wp", [HPG * HD, C], F32R, kind="ExternalInput")
    cosd = nc.dram_tensor("cosd", [96, N], F32, kind="ExternalInput")
    sind = nc.dram_tensor("sind", [96, N], F32, kind="ExternalInput")
    biasd = nc.dram_tensor("biasd", [128, CK], F32, kind="ExternalInput")
    outT = nc.dram_tensor("outT", [C, N], F32, kind="ExternalOutput")

    with tile.TileContext(nc) as tc:
        with tc.tile_pool(name="persist", bufs=1) as pp:
            qt_all = pp.tile([HD, HPG * N], F32R, name="qt_all")
            kt_all = pp.tile([HD, HPG * N], F32R, name="kt_all")
            v_tiles = [
                pp.tile([128, HPG, HD + 1], F32R, name=f"v{i}") for i in range(KTILES)
            ]
            vones_f = pp.tile([128, HPG], F32, name="vones_f")
            e_pool = [pp.tile([128, 2 * QS], F32R, tag="e_t", bufs=2, name=f"ep{i}")
                      for i in range(0)]  # tag reserved; tiles created in phase 2
            nc.vector.memset(vones_f[:], 1.0)

            # ================= phase 1: QKV + RoPE + repack =================
            with (
                tc.tile_pool(name="p1", bufs=1) as p1,
                tc.tile_pool(name="psum1", bufs=1, space="PSUM") as ps1,
            ):
                wqk_t = [p1.tile([128, 6 * 96], F32R, name=f"wqk{k}") for k in range(CK)]
                wv_t = [p1.tile([128, HPG * HD], F32R, name=f"wv{k}") for k in range(CK)]

                HS = N // 2  # half: RoPE-output/repack granularity
                for hn in range(2):
                    hs0 = hn * HS
                    # RoPE output tiles at half size (for big repack DMAs);
                    # QK psum stays per-quarter
                    rope_out = {
                        nm: p1.tile([96, HS], F32R, tag=nm, bufs=1, name=f"{nm}{hn}")
                        for nm in ("q_er", "q_or", "q_pr", "k_er", "k_or", "k_pr")
                    }

                    for sub in range(2):
                        qn = 2 * hn + sub
                        ts0 = qn * QS
                        sl = slice(sub * QS, (sub + 1) * QS)
                        xq = [
                            p1.tile([128, QS], F32R, tag=f"xq{k}", bufs=2,
                                    name=f"xq{k}_{qn}")
                            for k in range(CK)
                        ]
                        for k in range(CK):
                            nc.sync.dma_start(
                                xq[k][:], xT[k * 128:(k + 1) * 128, ts0:ts0 + QS]
                            )
                            if qn == 0:
                                # interleave weight loads with the first x
                                # quarter so early matmul inputs arrive first
                                nc.sync.dma_start(
                                    wqk_t[k][:], wqk[k * 128:(k + 1) * 128, :]
                                )
                                nc.sync.dma_start(
                                    wv_t[k][:], wv[k * 128:(k + 1) * 128, :]
                                )
                        cosq_t = p1.tile([96, QS], F32, tag="cosq", bufs=2, name=f"cosq{qn}")
                        sinq_t = p1.tile([96, QS], F32, tag="sinq", bufs=2, name=f"sinq{qn}")
                        nc.sync.dma_start(cosq_t[:], cosd[:, ts0:ts0 + QS])
                        nc.sync.dma_start(sinq_t[:], sind[:, ts0:ts0 + QS])
                        cosq = cosq_t[:]
                        sinq = sinq_t[:]

                        # V: out[t, d] for 4 t-tiles of 128 tokens
                        for tt in range(4):
                            v_ps = ps1.tile([128, HPG * HD], F32, tag="v_ps", bufs=2,
                                            name=f"vps{qn}_{tt}")
                            for k in range(CK):
                                nc.tensor.matmul(
                                    v_ps[:], xq[k][:, tt * 128:(tt + 1) * 128],
                                    wv_t[k][:],
                                    start=(k == 0), stop=(k == CK - 1),
                                )
                            vt = v_tiles[qn * 4 + tt]
                            nc.scalar.copy(
                                vt[:, :, 0:HD],
                                v_ps[:].rearrange("p (h d) -> p h d", h=HPG),
                            )
                            nc.scalar.copy(vt[:, :, HD], vones_f[:])

                        # QK blocks Q1 Q2 QP K1 K2 KP of 96 rows
                        qk_ps = []
                        for m in range(6):
                            ps = ps1.tile([96, QS], F32, tag="qk_ps", bufs=5,
                                          name=f"qkps{qn}_{m}")
                            for k in range(CK):
                                nc.tensor.matmul(
                                    ps[:], wqk_t[k][:, m * 96:(m + 1) * 96], xq[k][:],
                                    start=(k == 0), stop=(k == CK - 1),
                                )
                            qk_ps.append(ps)

                        def rope_pair(e_ps, o_ps, er, orr, tag):
                            t1 = p1.tile([96, QS], F32, tag="rtmpA", bufs=2,
                                         name=f"t1{tag}{qn}")
                            t2 = p1.tile([96, QS], F32, tag="rtmpB", bufs=2,
                                         name=f"t2{tag}{qn}")
                            nc.vector.tensor_tensor(t1[:], e_ps[:], cosq, ALU.mult)
                            nc.vector.tensor_tensor(t2[:], o_ps[:], sinq, ALU.mult)
                            nc.vector.tensor_tensor(er[:, sl], t1[:], t2[:], ALU.subtract)
                            t3 = p1.tile([96, QS], F32, tag="rtmpA", bufs=2,
                                         name=f"t3{tag}{qn}")
                            t4 = p1.tile([96, QS], F32, tag="rtmpB", bufs=2,
                                         name=f"t4{tag}{qn}")
                            nc.vector.tensor_tensor(t3[:], o_ps[:], cosq, ALU.mult)
                            nc.vector.tensor_tensor(t4[:], e_ps[:], sinq, ALU.mult)
                            nc.vector.tensor_tensor(orr[:, sl], t3[:], t4[:], ALU.add)

                        rope_pair(qk_ps[0], qk_ps[1], rope_out["q_er"], rope_out["q_or"], "q")
                        nc.scalar.copy(rope_out["q_pr"][:, sl], qk_ps[2][:])
                        rope_pair(qk_ps[3], qk_ps[4], rope_out["k_er"], rope_out["k_or"], "k")
                        nc.scalar.copy(rope_out["k_pr"][:, sl], qk_ps[5][:])

                    # repack into per-head [72, N]: rows 0-23 even, 24-47 odd,
                    # 48-71 pass; local head hh at cols [hh*N + hs0, ...)
                    for hh in range(HPG):
                        d0 = hh * N + hs0
                        for dst, src in (
                            (qt_all[0:24, d0:d0 + HS], rope_out["q_er"]),
                            (qt_all[24:48, d0:d0 + HS], rope_out["q_or"]),
                            (qt_all[48:72, d0:d0 + HS], rope_out["q_pr"]),
                            (kt_all[0:24, d0:d0 + HS], rope_out["k_er"]),
                            (kt_all[24:48, d0:d0 + HS], rope_out["k_or"]),
                            (kt_all[48:72, d0:d0 + HS], rope_out["k_pr"]),
                        ):
                            nc.sync.dma_start(dst, src[24 * hh:24 * hh + 24, :])

            # ================= phase 2+3: attention + projection =============
            # jq-outer / h-inner so the projection for token chunk jq overlaps
            # the attention of chunk jq+1. Exp batched over ST pairs to
            # amortize the ACTIVATE fixed overhead.
            with (
                tc.tile_pool(name="p2", bufs=1) as p2,
                tc.tile_pool(name="psum2", bufs=1, space="PSUM") as ps2,
            ):
                wp_t = [p2.tile([HD, C], F32R, name=f"wp{h}") for h in range(HPG)]
                bias_t = p2.tile([128, CK], F32, name="bias_t")
                nc.sync.dma_start(bias_t[:], biasd[:, :])
                for h in range(HPG):
                    nc.sync.dma_start(wp_t[h][:], wp[h * HD:(h + 1) * HD, :])

                ot_r = [p2.tile([HD, N], F32R, name=f"otr{h}") for h in range(HPG)]

                o_partial = {}

                def emit_proj(ct, jqp, mode="full"):
                    # mode "A": heads 0-1 only, park partial sum in SBUF
                    # mode "B": heads 2-3 + bias + parked partial, then store
                    heads = {"full": range(HPG), "A": range(2), "B": range(2, HPG)}[mode]
                    o_ps = ps2.tile([128, QS], F32, tag="o_ps", bufs=2,
                                    name=f"ops{ct}_{jqp}_{mode}")
                    for i, h in enumerate(heads):
                        nc.tensor.matmul(
                            o_ps[:],
                            wp_t[h][:, ct * 128:(ct + 1) * 128],
                            ot_r[h][:, jqp * QS:(jqp + 1) * QS],
                            start=(i == 0), stop=(i == len(heads) - 1),
                        )
                    if mode == "A":
                        part = p2.tile([128, QS], F32, tag=f"opart{ct}", bufs=1,
                                       name=f"opart{ct}")
                        nc.vector.tensor_copy(part[:], o_ps[:])
                        o_partial[ct] = part
                        return
                    o_sb = p2.tile([128, QS], F32, tag="o_sb", bufs=6,
                                   name=f"osb{ct}_{jqp}_{mode}")
                    if mode == "B":
                        nc.vector.scalar_tensor_tensor(
                            o_sb[:], o_ps[:], bias_t[:, ct:ct + 1], o_partial[ct][:],
                            ALU.add, ALU.add,
                        )
                    else:
                        nc.vector.tensor_scalar_add(o_sb[:], o_ps[:], bias_t[:, ct:ct + 1])
                    nc.sync.dma_start(
                        outT[ct * 128:(ct + 1) * 128, jqp * QS:(jqp + 1) * QS], o_sb[:]
                    )

                # pending projection groups, interleaved into the following
                # chunk's attention so proj matmuls fill PE gaps of the
                # ACT-bound inner loop
                pending = []

                for jq in range(NQ):
                    for h in range(HPG):
                        hb = h * N
                        ot_ps = ps2.tile([HD + 1, QS], F32, tag="ot_ps", bufs=2,
                                         name=f"otps{h}_{jq}")
                        for kp in range(KTILES // 2):
                            st_ps = ps2.tile([128, 2 * QS], F32, tag="st_ps", bufs=2,
                                             name=f"stps{h}_{jq}_{kp}")
                            for i in range(2):
                                kt = 2 * kp + i
                                nc.tensor.matmul(
                                    st_ps[:, i * QS:(i + 1) * QS],
                                    kt_all[:, hb + kt * 128: hb + (kt + 1) * 128],
                                    qt_all[:, hb + jq * QS: hb + (jq + 1) * QS],
                                    start=True, stop=True,
                                )
                            e_t = pp.tile([128, 2 * QS], F32R, tag="e_t", bufs=2,
                                          name=f"e{h}_{jq}_{kp}")
                            nc.scalar.activation(e_t[:], st_ps[:], AF.Exp, scale=SCALE)
                            for i in range(2):
                                kt = 2 * kp + i
                                nc.tensor.matmul(
                                    ot_ps[:], v_tiles[kt][:, h, :],
                                    e_t[:, i * QS:(i + 1) * QS],
                                    start=(kt == 0), stop=(kt == KTILES - 1),
                                )
                            if pending and (kp % 4 == 3):
                                jqp, ct, mode = pending.pop(0)
                                emit_proj(ct, jqp, mode)
                        ot_f = p2.tile([HD + 1, QS], F32, tag="otf", bufs=3,
                                       name=f"otf{h}_{jq}")
                        nc.vector.tensor_copy(ot_f[:], ot_ps[:])

                        # softmax denominator -> reciprocal, partition-parallel
                        den_sq = p2.tile([128, QS // 128], F32, tag="den_sq", bufs=4,
                                         name=f"den{h}_{jq}")
                        nc.sync.dma_start(den_sq[:], ot_f[HD:HD + 1, :])
                        rec_sq = p2.tile([128, QS // 128], F32, tag="rec_sq", bufs=4,
                                         name=f"recs{h}_{jq}")
                        nc.vector.reciprocal(rec_sq[:], den_sq[:])
                        rec_row = p2.tile([1, QS], F32, tag="rec_row", bufs=4,
                                          name=f"recrow{h}_{jq}")
                        nc.sync.dma_start(rec_row[:], rec_sq[:])
                        rec_bc = p2.tile([HD, QS], F32, tag="rec_bc", bufs=4,
                                         name=f"recbc{h}_{jq}")
                        nc.sync.dma_start(
                            rec_bc[:],
                            rec_row[0:1, :].unsqueeze(1).to_broadcast((1, HD, QS)),
                        )
                        nc.vector.tensor_tensor(
                            ot_r[h][:, jq * QS:(jq + 1) * QS],
                            ot_f[0:HD, :],
                            rec_bc[:], ALU.mult,
                        )
                        if jq == NQ - 1 and h == 1:
                            pending.extend((jq, ct, "A") for ct in range(CK))

                    if jq < NQ - 1:
                        pending.extend((jq, ct, "full") for ct in range(CK))

                for jqp, ct, mode in pending:
                    emit_proj(ct, jqp, mode)
                for ct in range(CK):
                    emit_proj(ct, NQ - 1, "B")

    bass_rust.generate_event_semaphores(nc)
    return nc


_NC = None


def _get_nc():
    global _NC
    if _NC is None:
        _NC = build_nc()
    return _NC


def kernel(x, Wqkv, Wproj, bproj, T, H, W):
    x = np.asarray(x, dtype=np.float32)
    Wqkv = np.asarray(Wqkv, dtype=np.float32)
    Wproj = np.asarray(Wproj, dtype=np.float32)
    bproj = np.asarray(bproj, dtype=np.float32)
    assert x.shape == (B, N, C) and Wqkv.shape == (C, 3 * C)
    assert (int(T), int(H), int(W)) == (GT, GH, GW)

    cos96, sin96 = _cos_sin_96()
    nc = _get_nc()

    in_maps = []
    for core in range(NCORES):
        b, g = divmod(core, HPG)
        heads = [HPG * g + i for i in range(HPG)]
        q_e = [h * HD + 2 * j for h in heads for j in range(24)]
        q_o = [h * HD + 2 * j + 1 for h in heads for j in range(24)]
        q_p = [h * HD + ROT + j for h in heads for j in range(24)]
        wqk_c = np.concatenate(
            [Wqkv[:, q_e], Wqkv[:, q_o], Wqkv[:, q_p],
             Wqkv[:, [C + i for i in q_e]], Wqkv[:, [C + i for i in q_o]],
             Wqkv[:, [C + i for i in q_p]]],
            axis=1,
        )
        wv_c = Wqkv[:, 2 * C + heads[0] * HD: 2 * C + (heads[-1] + 1) * HD]
        wp_c = Wproj[heads[0] * HD:(heads[-1] + 1) * HD, :]
        bias_c = bproj if g == 0 else np.zeros_like(bproj)
        in_maps.append({
            "xT": round_f32r(np.ascontiguousarray(x[b].T)),
            "wqk": round_f32r(wqk_c),
            "wv": round_f32r(np.ascontiguousarray(wv_c)),
            "wp": round_f32r(np.ascontiguousarray(wp_c)),
            "cosd": cos96,
            "sind": sin96,
            "biasd": np.ascontiguousarray(bias_c.reshape(CK, 128).T),
        })

    global _last_in_maps
    _last_in_maps = in_maps
    res = run_bass_kernel_spmd(nc, in_maps, core_ids=list(range(NCORES)))
    out = np.zeros((B, N, C), dtype=np.float32)
    for core in range(NCORES):
        b = core // HPG
        out[b] += res.results[core]["outT"].T
    return out



# revision 19
# speedup vs baseline: 1.1681x; 1.1681x over previous
"""Trainium2 Bass kernel for nn_Attention_79224966742132 (v2).

Dense transformer attention: QKV projection + axial RoPE + SDPA + output
projection for x (2, 2048, 1152), 16 heads of dim 72.

Sharding (8 cores): batch (2) x head-groups (4 heads/core). Each core:
full-x QKV for its 4 heads, RoPE, attention, row-parallel partial output
projection; host sums 4 partials per batch element.

Cost-model structure (matmul cost = out-free-size x cycles/row, fp8e4
DoubleRow = 0.5 c/r, LDWEIGHTS/contraction-depth free; every hwdge
dma_start holds the shared HWDGE device 625 ns -> DMA count minimized,
small repacks ride the Pool-engine software DGE instead):
- scores via fp8 DoubleRow: K cast to fp8 (duplicated pair slots), Q split
  hi+lo fp8 -> exact-ish Q x fp8 K at 0.5 cycles/row.
- exp on the scalar engine (~1.04 us per [128,1024] tile, the pipeline
  floor), output bf16 E tiles; score psum double-buffered via two tags.
- AV reoriented: stationary = E chunk [128k x 128q], moving = V [128k, 73]
  bf16 (ones column -> softmax denominator); out [q, d] psum, F=73.
- o normalized on DVE into column-packed [128,128] tiles, DMA-transposed
  into a 3-chunk [d, token] layout so the output projection contracts over
  3x128 packed chunks instead of 4x72 heads.
- K GEMM first, scores h-major, V/Q/AV/proj interleaved between score
  pairs to keep the exp stream fed.
"""
import math
import os
import sys

if "jax" not in sys.modules:
    _jp = os.environ.get("JAX_PLATFORMS")
    if _jp and "axon" not in _jp and "neuron" not in _jp:
        del os.environ["JAX_PLATFORMS"]

import numpy as np
import ml_dtypes

import bass_rust
import concourse.bass as bass
import concourse.mybir as mybir
import concourse.tile as tile
from concourse.bass_utils import run_bass_kernel_spmd

F32 = mybir.dt.float32
BF16 = mybir.dt.bfloat16
F8 = mybir.dt.float8e4
AF = mybir.ActivationFunctionType
ALU = mybir.AluOpType
PM = mybir.MatmulPerfMode

B = 2
N = 2048
C = 1152
NH = 16
HD = 72
HPG = 4
NCORES = 8
GT, GH, GW = 8, 16, 16
SCALE = 1.0 / math.sqrt(HD)
CK = C // 128          # 9 contraction chunks
QS = N // 4            # 512 token quarter
KTILES = N // 128      # 16 k tiles

# o_catT chunk packing: (chunk, row0, d0, d1) covering 4x72 dims in
# 3x128 rows; chunk2 rows 32:128 stay zero (wp rows there are zero).
NORM_MAP = [
    [(0, 0, 0, 72)],                       # h0 -> c0[0:72]
    [(0, 72, 0, 56), (1, 0, 56, 72)],      # h1 -> c0[72:128], c1[0:16]
    [(1, 16, 0, 72)],                      # h2 -> c1[16:88]
    [(1, 88, 0, 40), (2, 0, 40, 72)],      # h3 -> c1[88:128], c2[0:32]
]


def _axis_freqs(n: int) -> np.ndarray:
    base = np.linspace(1.0, 128.0, 8, dtype=np.float64) * np.pi
    pos = np.linspace(-1.0, 1.0, n, dtype=np.float64)
    return pos[:, None] * base[None, :]


def _cos_sin_96():
    f = np.zeros((GT, GH, GW, 24), dtype=np.float64)
    f[..., 0:8] = _axis_freqs(GT)[:, None, None, :]
    f[..., 8:16] = _axis_freqs(GH)[None, :, None, :]
    f[..., 16:24] = _axis_freqs(GW)[None, None, :, :]
    f = f.reshape(N, 24)
    cos24 = np.ascontiguousarray(np.cos(f).astype(np.float32).T)
    sin24 = np.ascontiguousarray(np.sin(f).astype(np.float32).T)
    return np.tile(cos24, (4, 1)), np.tile(sin24, (4, 1))   # (96, N)


def build_nc() -> bass.Bass:
    nc = bass.Bass()
    xT = nc.dram_tensor("xT", [C, N], BF16, kind="ExternalInput")
    wqk = nc.dram_tensor("wqk", [C, 6 * 96], BF16, kind="ExternalInput")
    wv = nc.dram_tensor("wv", [C, HPG * HD], BF16, kind="ExternalInput")
    wpd = nc.dram_tensor("wpd", [128, 3, C], BF16, kind="ExternalInput")
    cosd = nc.dram_tensor("cosd", [96, N], BF16, kind="ExternalInput")
    sind = nc.dram_tensor("sind", [96, N], BF16, kind="ExternalInput")
    identd = nc.dram_tensor("identd", [128, 128], BF16, kind="ExternalInput")
    outT = nc.dram_tensor("outT", [C, N], F32, kind="ExternalOutput")

    dma = nc.sync.dma_start
    pdma = nc.gpsimd.dma_start

    with tile.TileContext(nc) as tc:
        with (
            tc.tile_pool(name="pp", bufs=1) as pp,
            tc.tile_pool(name="psp", bufs=1, space="PSUM") as psp,
        ):
            # ---------------- persistent SBUF ----------------
            x_t = pp.tile([128, CK, N], BF16, name="x_t")
            wqk_t = pp.tile([128, CK, 6 * 96], BF16, name="wqk_t")
            wv_t = pp.tile([128, CK, HPG * HD], BF16, name="wv_t")
            wp_t = pp.tile([128, 3, C], BF16, name="wp_t")
            cos_t = pp.tile([96, N], BF16, name="cos_t")
            sin_t = pp.tile([96, N], BF16, name="sin_t")
            ident_t = pp.tile([128, 128], BF16, name="ident_t")
            # per-head packed fp8: kt8 [72, dup, kt, 128]; q8 [72, hi/lo, jq, 512]
            kt8 = [pp.tile([72, 2, KTILES, 128], F8, name=f"kt8_{h}")
                   for h in range(HPG)]
            q8 = [pp.tile([72, 2, 4, QS], F8, name=f"q8_{h}") for h in range(HPG)]
            v_t = [pp.tile([128, HPG, HD + 1], BF16, name=f"v{kt}")
                   for kt in range(KTILES)]
            o_n = [[pp.tile([128, 128], BF16, name=f"on{qt}_{c}") for c in range(3)]
                   for qt in range(4)]

            # one-time inits (Pool engine; SBUF only)
            for kt in range(KTILES):
                nc.gpsimd.memset(v_t[kt][:, :, HD], 1.0)
            for qt in range(4):
                nc.gpsimd.memset(o_n[qt][2][:, 32:128], 0.0)
                nc.gpsimd.memset(o_n[qt][2][:, 32:33], 1.0)

            # ---------------- input loads (few, large DMAs) ----------------
            def kp(ap):
                return ap.rearrange("(k p) c -> p k c", p=128)

            dma(wqk_t[:, :, 288:576], kp(wqk[:, 288:576]))
            for s3 in range(3):
                dma(x_t[:, 3 * s3:3 * s3 + 3, 0:QS],
                    kp(xT[:, 0:QS])[:, 3 * s3:3 * s3 + 3, :])
            dma(cos_t[:], cosd[:, :])
            dma(sin_t[:], sind[:, :])
            dma(wqk_t[:, :, 0:288], kp(wqk[:, 0:288]))
            dma(x_t[:, :, QS:2 * QS], kp(xT[:, QS:2 * QS]))

            def late_loads():
                for q in range(2, 4):
                    dma(x_t[:, :, q * QS:(q + 1) * QS],
                        kp(xT[:, q * QS:(q + 1) * QS]))
                dma(wv_t[:], kp(wv[:, :]))
                dma(wp_t[:], wpd[:, :, :])
                dma(ident_t[:], identd[:, :])

            # ---------------- unit emitters ----------------
            uid = [0]

            def nid(p):
                uid[0] += 1
                return f"{p}{uid[0]}"

            def kq_units(q, which):
                """3 units: e-block; o-block+rope; p-block+cast+repack."""
                ts0 = q * QS
                sl = slice(ts0, ts0 + QS)
                m0 = 3 if which == "K" else 0
                st = {}

                def blk(bi):
                    p = psp.tile([96, QS], F32, tag="ps5", bufs=2, name=nid("qk"))
                    m = m0 + bi
                    for k in range(CK):
                        nc.tensor.matmul(
                            p[:], wqk_t[:, k, m * 96:(m + 1) * 96],
                            x_t[:, k, sl], start=(k == 0), stop=(k == CK - 1),
                        )
                    return p

                def u1():
                    st["e"] = blk(0)

                def u2():
                    st["o"] = blk(1)
                    e_ps, o_ps = st["e"], st["o"]
                    t1 = pp.tile([96, QS], F32, tag="rt1", bufs=1, name=nid("t1"))
                    t4 = pp.tile([96, QS], F32, tag="rt4", bufs=1, name=nid("t4"))
                    # e-psum consumers first so the p-block can reuse its slot
                    nc.vector.tensor_tensor(t1[:], e_ps[:], cos_t[:, sl], ALU.mult)
                    nc.vector.tensor_tensor(t4[:], e_ps[:], sin_t[:, sl], ALU.mult)
                    t2 = pp.tile([96, QS], F32, tag="rt2", bufs=1, name=nid("t2"))
                    t3 = pp.tile([96, QS], F32, tag="rt3", bufs=1, name=nid("t3"))
                    nc.vector.tensor_tensor(t2[:], o_ps[:], sin_t[:, sl], ALU.mult)
                    nc.vector.tensor_tensor(t3[:], o_ps[:], cos_t[:, sl], ALU.mult)
                    st["t"] = (t1, t2, t3, t4)

                def u3():
                    p_ps = blk(2)
                    t1, t2, t3, t4 = st["t"]
                    if which == "K":
                        sk = pp.tile([96, 3, QS], F8, tag="sk", bufs=2,
                                     name=nid("sk"))
                        nc.vector.tensor_tensor(sk[:, 0, :], t1[:], t2[:],
                                                ALU.subtract)
                        nc.vector.tensor_tensor(sk[:, 1, :], t3[:], t4[:], ALU.add)
                        nc.vector.tensor_copy(sk[:, 2, :], p_ps[:])
                        for h in range(HPG):
                            for bi in range(3):
                                eng = pdma if (h * 3 + bi) % 2 == 0 else dma
                                eng(kt8[h][24 * bi:24 * bi + 24, :,
                                           4 * q:4 * q + 4, :],
                                    sk[24 * h:24 * h + 24, bi, :].unsqueeze(1)
                                    .to_broadcast((24, 2, QS)))
                    else:
                        erf = pp.tile([96, QS], F32, tag="qer", bufs=1,
                                      name=nid("qer"))
                        orf = pp.tile([96, QS], F32, tag="qor", bufs=1,
                                      name=nid("qor"))
                        nc.vector.tensor_tensor(erf[:], t1[:], t2[:], ALU.subtract)
                        nc.vector.tensor_tensor(orf[:], t3[:], t4[:], ALU.add)
                        sq = pp.tile([96, 3, 2, QS], F8, tag="sq", bufs=2,
                                     name=nid("sq"))
                        for bi, srcf in ((0, erf), (1, orf), (2, p_ps)):
                            nc.vector.tensor_copy(sq[:, bi, 0, :], srcf[:])
                            nc.vector.tensor_tensor(sq[:, bi, 1, :], srcf[:],
                                                    sq[:, bi, 0, :], ALU.subtract)
                        for h in range(HPG):
                            for bi in range(3):
                                eng = pdma if (h * 3 + bi) % 2 == 0 else dma
                                eng(q8[h][24 * bi:24 * bi + 24, :, q, :],
                                    sq[24 * h:24 * h + 24, bi, :, :])

                return [u1, u2, u3]

            def emit_kq(q, which):
                for u in kq_units(q, which):
                    u()

            e_store = {}
            pair_n = [0]

            def emit_pair(h, jq, p):
                """Scores for kt pair (2p, 2p+1) via fp8 DoubleRow + exp."""
                tag = "stA" if pair_n[0] % 2 == 0 else "stB"
                pair_n[0] += 1
                st = psp.tile([128, 2 * QS], F32, tag=tag, bufs=1, name=nid("st"))
                for i in range(2):
                    kt = 2 * p + i
                    nc.tensor.matmul(
                        st[:, i * QS:(i + 1) * QS],
                        kt8[h][:, :, kt, :], q8[h][:, :, jq, :],
                        start=True, stop=True, perf_mode=PM.DoubleRow,
                    )
                e_t = pp.tile([128, 2 * QS], BF16, tag="eA", bufs=28, name=nid("e"))
                nc.scalar.activation(e_t[:], st[:], AF.Exp, scale=SCALE)
                for i in range(2):
                    e_store[(jq, h, 2 * p + i)] = (e_t, i * QS)

            def v_unit(kt):
                vp = psp.tile([128, QS], F32, tag="ps5", bufs=2, name=nid("vp"))
                for k in range(CK):
                    nc.tensor.matmul(
                        vp[:, 0:HPG * HD],
                        x_t[:, k, kt * 128:(kt + 1) * 128], wv_t[:, k, :],
                        start=(k == 0), stop=(k == CK - 1),
                    )
                nc.vector.tensor_copy(
                    v_t[kt][:, :, 0:HD],
                    vp[:, 0:HPG * HD].rearrange("p (h d) -> p h d", h=HPG),
                )

            o_cat = {}

            def av_block(h, jq):
                """AV for all 4 q-subtiles of (h, jq), kt-major so E tiles
                release early and all subtiles finish with the last exp."""
                ot = psp.tile([128, 4, 128], F32, tag="ot", bufs=1, name=nid("ot"))
                for qt in range(4):
                    for kt in range(KTILES):
                        e_t, off = e_store[(jq, h, kt)]
                        nc.tensor.matmul(
                            ot[:, qt, 0:HD + 1],
                            e_t[:, off + qt * 128: off + (qt + 1) * 128],
                            v_t[kt][:, h, :],
                            start=(kt == 0), stop=(kt == KTILES - 1),
                        )
                if h == HPG - 1:
                    o_cat[jq] = pp.tile([128, 3, QS], BF16, tag="ocat",
                                        bufs=2, name=nid("oc"))
                for qt in range(4):
                    rec = pp.tile([128, 1], F32, tag="rec", bufs=4, name=nid("rec"))
                    nc.vector.reciprocal(rec[:], ot[:, qt, HD:HD + 1])
                    for (c, r0, d0, d1) in NORM_MAP[h]:
                        nc.vector.tensor_scalar_mul(
                            o_n[qt][c][:, r0:r0 + (d1 - d0)],
                            ot[:, qt, d0:d1], rec[:],
                        )
                    if h == HPG - 1:
                        tp_ps = psp.tile([128, 3, 128], BF16, tag="tpp",
                                         bufs=1, name=nid("tp"))
                        for c in range(3):
                            nc.tensor.transpose(tp_ps[:, c, :], o_n[qt][c][:],
                                                ident_t[:])
                        nc.vector.tensor_copy(
                            o_cat[jq][:, :, qt * 128:(qt + 1) * 128], tp_ps[:])

            def emit_proj(jq, g, tail=False):
                """Projection for ct chunk triple g; bias rides row 32 of
                o_cat chunk 2 (ones) x wp chunk-2 bias row."""
                oc = o_cat[jq]
                osb = pp.tile([128, 3, QS], F32, tag="osb", bufs=2, name=nid("osb"))
                for j in range(3):
                    ct = 3 * g + j
                    tag = "ot" if tail and (3 * g + j) % 2 else "ps5"
                    pj = psp.tile([128, QS], F32, tag=tag,
                                  bufs=1 if tag == "ot" else 2, name=nid("pj"))
                    for c in range(3):
                        nc.tensor.matmul(
                            pj[:], wp_t[:, c, ct * 128:(ct + 1) * 128], oc[:, c, :],
                            start=(c == 0), stop=(c == 2),
                        )
                    if tail:
                        nc.scalar.copy(osb[:, j, :], pj[:])
                        dma(outT[ct * 128:(ct + 1) * 128,
                                 jq * QS:(jq + 1) * QS], osb[:, j, :])
                    else:
                        nc.vector.tensor_copy(osb[:, j, :], pj[:])
                if not tail:
                    dma(outT[g * 384:(g + 1) * 384, jq * QS:(jq + 1) * QS]
                        .rearrange("(c p) t -> p c t", p=128), osb[:])

            # ---------------- emission schedule ----------------
            fill = []

            def pop_fill(budget):
                spent = 0
                while fill and spent < budget:
                    cyc, fn = fill.pop(0)
                    fn()
                    spent += cyc

            def push_av(jq, h):
                fill.append((5000, (lambda jj=jq, hh=h: av_block(hh, jj))))

            emit_kq(0, "K")
            emit_kq(0, "Q")

            # jq0/h0: K quarters + Q1 sub-units woven between the 8 pairs
            h0_units = (kq_units(1, "K") + kq_units(2, "K") + kq_units(3, "K")
                        + kq_units(1, "Q"))
            # after pair p emit units [ .. ]: K(q) complete before pairs 2q..
            h0_sched = [[0], [1, 2], [3], [4, 5], [6], [7, 8], [9], [10, 11]]
            for p in range(8):
                emit_pair(0, 0, p)
                for ui in h0_sched[p]:
                    h0_units[ui]()
                if p == 1:
                    late_loads()
            fill.extend((2700, (lambda kk=kt: v_unit(kk))) for kt in range(KTILES))

            slots = [(jq, h) for jq in range(4) for h in range(HPG)]
            for si, (jq, h) in enumerate(slots):
                if jq == 0 and h == 0:
                    continue
                if h == 0 and jq < 3:
                    fill[0:0] = [(4700, u) for u in kq_units(jq + 1, "Q")]
                for p in range(8):
                    emit_pair(h, jq, p)
                    if jq == 0:
                        if p % 2 == 1:
                            pop_fill(2600)
                    else:
                        pop_fill(1100 if len(fill) < 14 else 2600)
                # AV lags two head-slots so its exps are fully drained
                if si >= 2:
                    push_av(*slots[si - 2])
                if h == 1 and jq >= 1:
                    fill.extend(
                        (4800, (lambda jj=jq - 1, gg=g: emit_proj(jj, gg)))
                        for g in range(3))
            push_av(3, HPG - 2)
            push_av(3, HPG - 1)
            fill.extend((4800, (lambda gg=g: emit_proj(3, gg, tail=True)))
                        for g in range(3))
            while fill:
                _, fn = fill.pop(0)
                fn()

    bass_rust.generate_event_semaphores(nc)
    return nc


_NC = None


def _get_nc():
    global _NC
    if _NC is None:
        _NC = build_nc()
    return _NC


def kernel(x, Wqkv, Wproj, bproj, T, H, W):
    x = np.asarray(x, dtype=np.float32)
    Wqkv = np.asarray(Wqkv, dtype=np.float32)
    Wproj = np.asarray(Wproj, dtype=np.float32)
    bproj = np.asarray(bproj, dtype=np.float32)
    assert x.shape == (B, N, C) and Wqkv.shape == (C, 3 * C)
    assert (int(T), int(H), int(W)) == (GT, GH, GW)

    cos96, sin96 = _cos_sin_96()
    bf = ml_dtypes.bfloat16
    nc = _get_nc()

    in_maps = []
    for core in range(NCORES):
        b, g = divmod(core, HPG)
        heads = [HPG * g + i for i in range(HPG)]
        q_e = [h * HD + 2 * j for h in heads for j in range(24)]
        q_o = [h * HD + 2 * j + 1 for h in heads for j in range(24)]
        q_p = [h * HD + 48 + j for h in heads for j in range(24)]
        wqk_c = np.concatenate(
            [Wqkv[:, q_e], Wqkv[:, q_o], Wqkv[:, q_p],
             Wqkv[:, [C + i for i in q_e]], Wqkv[:, [C + i for i in q_o]],
             Wqkv[:, [C + i for i in q_p]]],
            axis=1,
        )
        wv_c = Wqkv[:, 2 * C + heads[0] * HD: 2 * C + (heads[-1] + 1) * HD]
        # packed wp rows per NORM_MAP chunk layout
        wp_c = np.zeros((128, 3, C), dtype=np.float32)
        for h in range(HPG):
            gr = heads[h] * HD
            for (c, r0, d0, d1) in NORM_MAP[h]:
                wp_c[r0:r0 + (d1 - d0), c, :] = Wproj[gr + d0:gr + d1, :]
        if g == 0:
            wp_c[32, 2, :] = bproj
        in_maps.append({
            "xT": np.ascontiguousarray(x[b].T).astype(bf),
            "wqk": wqk_c.astype(bf),
            "wv": np.ascontiguousarray(wv_c).astype(bf),
            "wpd": wp_c.astype(bf),
            "cosd": cos96.astype(bf),
            "sind": sin96.astype(bf),
            "identd": np.eye(128, dtype=np.float32).astype(bf),
        })

    res = run_bass_kernel_spmd(nc, in_maps, core_ids=list(range(NCORES)))
    global _last_res
    _last_res = res
    out = np.zeros((B, N, C), dtype=np.float32)
    for core in range(NCORES):
        b = core // HPG
        out[b] += res.results[core]["outT"].T
    return out


# revision 27
# speedup vs baseline: 1.2288x; 1.0519x over previous
"""Trainium2 Bass kernel for nn_Attention_79224966742132 (v2).

Dense transformer attention: QKV projection + axial RoPE + SDPA + output
projection for x (2, 2048, 1152), 16 heads of dim 72.

Sharding (8 cores): batch (2) x head-groups (4 heads/core). Each core:
full-x QKV for its 4 heads, RoPE, attention, row-parallel partial output
projection; host sums 4 partials per batch element.

Cost-model structure (matmul cost = out-free-size x cycles/row, fp8e4
DoubleRow = 0.5 c/r, LDWEIGHTS/contraction-depth free; every hwdge
dma_start holds the shared HWDGE device 625 ns -> DMA count minimized,
small repacks ride the Pool-engine software DGE instead):
- scores via fp8 DoubleRow: K cast to fp8 (duplicated pair slots), Q split
  hi+lo fp8 -> exact-ish Q x fp8 K at 0.5 cycles/row.
- exp on the scalar engine (~1.04 us per [128,1024] tile, the pipeline
  floor), output bf16 E tiles; score psum double-buffered via two tags.
- AV reoriented: stationary = E chunk [128k x 128q], moving = V [128k, 73]
  bf16 (ones column -> softmax denominator); out [q, d] psum, F=73.
- o normalized on DVE into column-packed [128,128] tiles, DMA-transposed
  into a 3-chunk [d, token] layout so the output projection contracts over
  3x128 packed chunks instead of 4x72 heads.
- K GEMM first, scores h-major, V/Q/AV/proj interleaved between score
  pairs to keep the exp stream fed.
"""
import math
import os
import sys

if "jax" not in sys.modules:
    _jp = os.environ.get("JAX_PLATFORMS")
    if _jp and "axon" not in _jp and "neuron" not in _jp:
        del os.environ["JAX_PLATFORMS"]

import numpy as np
import ml_dtypes

import bass_rust
import concourse.bass as bass
import concourse.mybir as mybir
import concourse.tile as tile
from concourse.bass_utils import run_bass_kernel_spmd

F32 = mybir.dt.float32
BF16 = mybir.dt.bfloat16
F8 = mybir.dt.float8e4
AF = mybir.ActivationFunctionType
ALU = mybir.AluOpType
PM = mybir.MatmulPerfMode

B = 2
N = 2048
C = 1152
NH = 16
HD = 72
HPG = 4
NCORES = 8
GT, GH, GW = 8, 16, 16
SCALE = 1.0 / math.sqrt(HD)
CK = C // 128          # 9 contraction chunks
QS = N // 4            # 512 token quarter
KTILES = N // 128      # 16 k tiles

# o_catT chunk packing: (chunk, row0, d0, d1) covering 4x72 dims in
# 3x128 rows; chunk2 rows 32:128 stay zero (wp rows there are zero).
NORM_MAP = [
    [(0, 0, 0, 72)],                       # h0 -> c0[0:72]
    [(0, 72, 0, 56), (1, 0, 56, 72)],      # h1 -> c0[72:128], c1[0:16]
    [(1, 16, 0, 72)],                      # h2 -> c1[16:88]
    [(1, 88, 0, 40), (2, 0, 40, 72)],      # h3 -> c1[88:128], c2[0:32]
]


def _axis_freqs(n: int) -> np.ndarray:
    base = np.linspace(1.0, 128.0, 8, dtype=np.float64) * np.pi
    pos = np.linspace(-1.0, 1.0, n, dtype=np.float64)
    return pos[:, None] * base[None, :]


def _cos_sin_96():
    f = np.zeros((GT, GH, GW, 24), dtype=np.float64)
    f[..., 0:8] = _axis_freqs(GT)[:, None, None, :]
    f[..., 8:16] = _axis_freqs(GH)[None, :, None, :]
    f[..., 16:24] = _axis_freqs(GW)[None, None, :, :]
    f = f.reshape(N, 24)
    cos24 = np.ascontiguousarray(np.cos(f).astype(np.float32).T)
    sin24 = np.ascontiguousarray(np.sin(f).astype(np.float32).T)
    return np.tile(cos24, (4, 1)), np.tile(sin24, (4, 1))   # (96, N)


def build_nc() -> bass.Bass:
    nc = bass.Bass()
    xT = nc.dram_tensor("xT", [C, N], BF16, kind="ExternalInput")
    wqk = nc.dram_tensor("wqk", [C, 6 * 96], BF16, kind="ExternalInput")
    wv = nc.dram_tensor("wv", [C, HPG * HD], BF16, kind="ExternalInput")
    wpd = nc.dram_tensor("wpd", [128, 3, C], BF16, kind="ExternalInput")
    cosd = nc.dram_tensor("cosd", [96, N], BF16, kind="ExternalInput")
    sind = nc.dram_tensor("sind", [96, N], BF16, kind="ExternalInput")
    identd = nc.dram_tensor("identd", [128, 128], BF16, kind="ExternalInput")
    outT = nc.dram_tensor("outT", [C, N], F32, kind="ExternalOutput")

    dma = nc.sync.dma_start
    pdma = nc.gpsimd.dma_start

    with tile.TileContext(nc) as tc:
        with (
            tc.tile_pool(name="pp", bufs=1) as pp,
            tc.tile_pool(name="psp", bufs=1, space="PSUM") as psp,
        ):
            # ---------------- persistent SBUF ----------------
            x_t = pp.tile([128, CK, N], BF16, name="x_t")
            wqk_t = pp.tile([128, CK, 6 * 96], BF16, name="wqk_t")
            wv_t = pp.tile([128, CK, HPG * HD], BF16, name="wv_t")
            wp_t = pp.tile([128, 3, C], BF16, name="wp_t")
            cos_t = pp.tile([96, N], BF16, name="cos_t")
            sin_t = pp.tile([96, N], BF16, name="sin_t")
            ident_t = pp.tile([128, 128], BF16, name="ident_t")
            # per-head packed fp8: kt8 [72, dup, kt, 128]; q8 [72, hi/lo, jq, 512]
            kt8 = [pp.tile([72, 2, KTILES, 128], F8, name=f"kt8_{h}")
                   for h in range(HPG)]
            q8 = [pp.tile([72, 2, 4, QS], F8, name=f"q8_{h}") for h in range(HPG)]
            v_t = [pp.tile([128, HPG, HD + 1], BF16, name=f"v{kt}")
                   for kt in range(KTILES)]
            o_n = [[pp.tile([128, 128], BF16, name=f"on{qt}_{c}") for c in range(3)]
                   for qt in range(4)]

            # one-time inits (Pool engine; SBUF only)
            for kt in range(KTILES):
                nc.gpsimd.memset(v_t[kt][:, :, HD], 1.0)
            for qt in range(4):
                nc.gpsimd.memset(o_n[qt][2][:, 32:128], 0.0)
                nc.gpsimd.memset(o_n[qt][2][:, 32:33], 1.0)

            # ---------------- input loads (few, large DMAs) ----------------
            def kp(ap):
                return ap.rearrange("(k p) c -> p k c", p=128)

            dma(wqk_t[:, :, 288:576], kp(wqk[:, 288:576]))
            for s3 in range(3):
                dma(x_t[:, 3 * s3:3 * s3 + 3, 0:QS],
                    kp(xT[:, 0:QS])[:, 3 * s3:3 * s3 + 3, :])
            dma(cos_t[:], cosd[:, :])
            dma(sin_t[:], sind[:, :])
            dma(wqk_t[:, :, 0:288], kp(wqk[:, 0:288]))
            dma(x_t[:, :, QS:2 * QS], kp(xT[:, QS:2 * QS]))

            def late_loads():
                for q in range(2, 4):
                    dma(x_t[:, :, q * QS:(q + 1) * QS],
                        kp(xT[:, q * QS:(q + 1) * QS]))
                dma(wv_t[:], kp(wv[:, :]))
                dma(wp_t[:], wpd[:, :, :])
                dma(ident_t[:], identd[:, :])

            # ---------------- unit emitters ----------------
            uid = [0]

            def nid(p):
                uid[0] += 1
                return f"{p}{uid[0]}"

            def kq_units(q, which, act_cast=False):
                """3 units: e-block; o-block+rope; p-block+cast+repack.
                act_cast: route fp8 casts to the (idle) scalar engine."""
                ts0 = q * QS
                sl = slice(ts0, ts0 + QS)
                m0 = 3 if which == "K" else 0
                st = {}

                def blk(bi):
                    p = psp.tile([96, QS], F32, tag="ps5", bufs=2, name=nid("qk"))
                    m = m0 + bi
                    for k in range(CK):
                        nc.tensor.matmul(
                            p[:], wqk_t[:, k, m * 96:(m + 1) * 96],
                            x_t[:, k, sl], start=(k == 0), stop=(k == CK - 1),
                        )
                    return p

                def u1():
                    st["e"] = blk(0)

                def u2():
                    st["o"] = blk(1)
                    e_ps, o_ps = st["e"], st["o"]
                    t1 = pp.tile([96, QS], F32, tag="rt1", bufs=1, name=nid("t1"))
                    t4 = pp.tile([96, QS], F32, tag="rt4", bufs=1, name=nid("t4"))
                    # e-psum consumers first so the p-block can reuse its slot
                    nc.vector.tensor_tensor(t1[:], e_ps[:], cos_t[:, sl], ALU.mult)
                    nc.vector.tensor_tensor(t4[:], e_ps[:], sin_t[:, sl], ALU.mult)
                    t2 = pp.tile([96, QS], F32, tag="rt2", bufs=1, name=nid("t2"))
                    t3 = pp.tile([96, QS], F32, tag="rt3", bufs=1, name=nid("t3"))
                    nc.vector.tensor_tensor(t2[:], o_ps[:], sin_t[:, sl], ALU.mult)
                    nc.vector.tensor_tensor(t3[:], o_ps[:], cos_t[:, sl], ALU.mult)
                    st["t"] = (t1, t2, t3, t4)

                def u3():
                    p_ps = blk(2)
                    t1, t2, t3, t4 = st["t"]
                    if which == "K":
                        sk = pp.tile([96, 3, QS], F8, tag="sk", bufs=2,
                                     name=nid("sk"))
                        nc.vector.tensor_tensor(sk[:, 0, :], t1[:], t2[:],
                                                ALU.subtract)
                        nc.vector.tensor_tensor(sk[:, 1, :], t3[:], t4[:], ALU.add)
                        if act_cast:
                            nc.scalar.copy(sk[:, 2, :], p_ps[:])
                        else:
                            nc.vector.tensor_copy(sk[:, 2, :], p_ps[:])
                        for h in range(HPG):
                            for bi in range(3):
                                eng = pdma if (h * 3 + bi) % 2 == 0 else dma
                                eng(kt8[h][24 * bi:24 * bi + 24, :,
                                           4 * q:4 * q + 4, :],
                                    sk[24 * h:24 * h + 24, bi, :].unsqueeze(1)
                                    .to_broadcast((24, 2, QS)))
                    else:
                        erf = pp.tile([96, QS], F32, tag="qer", bufs=1,
                                      name=nid("qer"))
                        orf = pp.tile([96, QS], F32, tag="qor", bufs=1,
                                      name=nid("qor"))
                        nc.vector.tensor_tensor(erf[:], t1[:], t2[:], ALU.subtract)
                        nc.vector.tensor_tensor(orf[:], t3[:], t4[:], ALU.add)
                        sq = pp.tile([96, 3, 2, QS], F8, tag="sq", bufs=2,
                                     name=nid("sq"))
                        for bi, srcf in ((0, erf), (1, orf), (2, p_ps)):
                            if act_cast:
                                nc.scalar.copy(sq[:, bi, 0, :], srcf[:])
                            else:
                                nc.vector.tensor_copy(sq[:, bi, 0, :], srcf[:])
                            nc.vector.tensor_tensor(sq[:, bi, 1, :], srcf[:],
                                                    sq[:, bi, 0, :], ALU.subtract)
                        for h in range(HPG):
                            for bi in range(3):
                                eng = pdma if (h * 3 + bi) % 2 == 0 else dma
                                eng(q8[h][24 * bi:24 * bi + 24, :, q, :],
                                    sq[24 * h:24 * h + 24, bi, :, :])

                return [u1, u2, u3]

            def emit_kq(q, which, act_cast=False):
                for u in kq_units(q, which, act_cast):
                    u()

            e_store = {}
            pair_n = [0]

            def emit_pair(h, jq, p):
                """Scores for kt pair (2p, 2p+1) via fp8 DoubleRow + exp."""
                tag = "stA" if pair_n[0] % 2 == 0 else "stB"
                pair_n[0] += 1
                st = psp.tile([128, 2 * QS], F32, tag=tag, bufs=1, name=nid("st"))
                for i in range(2):
                    kt = 2 * p + i
                    nc.tensor.matmul(
                        st[:, i * QS:(i + 1) * QS],
                        kt8[h][:, :, kt, :], q8[h][:, :, jq, :],
                        start=True, stop=True, perf_mode=PM.DoubleRow,
                    )
                e_t = pp.tile([128, 2 * QS], BF16, tag="eA", bufs=28, name=nid("e"))
                nc.scalar.activation(e_t[:], st[:], AF.Exp, scale=SCALE)
                for i in range(2):
                    e_store[(jq, h, 2 * p + i)] = (e_t, i * QS)

            def v_unit(kt):
                vp = psp.tile([128, QS], F32, tag="ps5", bufs=2, name=nid("vp"))
                for k in range(CK):
                    nc.tensor.matmul(
                        vp[:, 0:HPG * HD],
                        x_t[:, k, kt * 128:(kt + 1) * 128], wv_t[:, k, :],
                        start=(k == 0), stop=(k == CK - 1),
                    )
                nc.vector.tensor_copy(
                    v_t[kt][:, :, 0:HD],
                    vp[:, 0:HPG * HD].rearrange("p (h d) -> p h d", h=HPG),
                )

            o_cat = {}

            def av_block(h, jq):
                """AV for all 4 q-subtiles of (h, jq), kt-major so E tiles
                release early and all subtiles finish with the last exp."""
                ot = psp.tile([128, 4, 128], F32, tag="ot", bufs=1, name=nid("ot"))
                for qt in range(4):
                    for kt in range(KTILES):
                        e_t, off = e_store[(jq, h, kt)]
                        nc.tensor.matmul(
                            ot[:, qt, 0:HD + 1],
                            e_t[:, off + qt * 128: off + (qt + 1) * 128],
                            v_t[kt][:, h, :],
                            start=(kt == 0), stop=(kt == KTILES - 1),
                        )
                if h == HPG - 1:
                    o_cat[jq] = pp.tile([128, 3, QS], BF16, tag="ocat",
                                        bufs=2, name=nid("oc"))
                for qt in range(4):
                    rec = pp.tile([128, 1], F32, tag="rec", bufs=4, name=nid("rec"))
                    nc.vector.reciprocal(rec[:], ot[:, qt, HD:HD + 1])
                    for (c, r0, d0, d1) in NORM_MAP[h]:
                        nc.vector.tensor_scalar_mul(
                            o_n[qt][c][:, r0:r0 + (d1 - d0)],
                            ot[:, qt, d0:d1], rec[:],
                        )
                    if h == HPG - 1:
                        tp_ps = psp.tile([128, 3, 128], BF16, tag="tpp",
                                         bufs=1, name=nid("tp"))
                        for c in range(3):
                            nc.tensor.transpose(tp_ps[:, c, :], o_n[qt][c][:],
                                                ident_t[:])
                        nc.vector.tensor_copy(
                            o_cat[jq][:, :, qt * 128:(qt + 1) * 128], tp_ps[:])

            def emit_proj(jq, g, tail=False):
                """Projection for ct chunk triple g; bias rides row 32 of
                o_cat chunk 2 (ones) x wp chunk-2 bias row."""
                oc = o_cat[jq]
                osb = pp.tile([128, 3, QS], F32, tag="osb", bufs=2, name=nid("osb"))
                for j in range(3):
                    ct = 3 * g + j
                    tag = "ot" if tail and (3 * g + j) % 2 else "ps5"
                    pj = psp.tile([128, QS], F32, tag=tag,
                                  bufs=1 if tag == "ot" else 2, name=nid("pj"))
                    for c in range(3):
                        nc.tensor.matmul(
                            pj[:], wp_t[:, c, ct * 128:(ct + 1) * 128], oc[:, c, :],
                            start=(c == 0), stop=(c == 2),
                        )
                    if tail:
                        if j % 2 == 0:
                            nc.scalar.copy(osb[:, j, :], pj[:])
                        else:
                            nc.vector.tensor_copy(osb[:, j, :], pj[:])
                        dma(outT[ct * 128:(ct + 1) * 128,
                                 jq * QS:(jq + 1) * QS], osb[:, j, :])
                    else:
                        nc.vector.tensor_copy(osb[:, j, :], pj[:])
                if not tail:
                    dma(outT[g * 384:(g + 1) * 384, jq * QS:(jq + 1) * QS]
                        .rearrange("(c p) t -> p c t", p=128), osb[:])

            # ---------------- emission schedule ----------------
            fill = []

            def pop_fill(budget):
                spent = 0
                while fill and spent < budget:
                    cyc, fn = fill.pop(0)
                    fn()
                    spent += cyc

            def push_av(jq, h):
                fill.append((5000, (lambda jj=jq, hh=h: av_block(hh, jj))))

            emit_kq(0, "K", act_cast=True)
            emit_kq(0, "Q", act_cast=True)

            # jq0/h0: K quarters + Q1 sub-units woven between the 8 pairs
            h0_units = (kq_units(1, "K", True) + kq_units(2, "K", True)
                        + kq_units(3, "K", True) + kq_units(1, "Q", True))
            # after pair p emit units [ .. ]: K(q) complete before pairs 2q..
            h0_sched = [[0], [1, 2], [3], [4, 5], [6], [7, 8], [9], [10, 11]]
            for p in range(8):
                emit_pair(0, 0, p)
                for ui in h0_sched[p]:
                    h0_units[ui]()
                if p == 1:
                    late_loads()
            fill.extend((2700, (lambda kk=kt: v_unit(kk))) for kt in range(10))

            slots = [(jq, h) for jq in range(4) for h in range(HPG)]
            for si, (jq, h) in enumerate(slots):
                if jq == 0 and h == 0:
                    continue
                if h == 0 and jq < 3:
                    fill.extend((4700, u) for u in kq_units(jq + 1, "Q"))
                for p in range(8):
                    emit_pair(h, jq, p)
                    if jq == 0 and h < HPG - 1:
                        if p % 2 == 1:
                            pop_fill(2600)
                    else:
                        pop_fill(1100 if len(fill) < 14 else 2600)
                if jq == 0 and h == HPG - 2:
                    # deferred V tiles must be emitted before AV(jq0, h0) pops
                    fill.extend((2700, (lambda kk=kt: v_unit(kk)))
                                for kt in range(10, KTILES))
                # AV lags two head-slots so its exps are fully drained
                if si >= 2:
                    push_av(*slots[si - 2])
                if h == 1 and jq >= 1:
                    fill.extend(
                        (4800, (lambda jj=jq - 1, gg=g: emit_proj(jj, gg)))
                        for g in range(3))
            push_av(3, HPG - 2)
            push_av(3, HPG - 1)
            fill.extend((4800, (lambda gg=g: emit_proj(3, gg, tail=True)))
                        for g in range(3))
            while fill:
                _, fn = fill.pop(0)
                fn()

    bass_rust.generate_event_semaphores(nc)
    return nc


_NC = None


def _get_nc():
    global _NC
    if _NC is None:
        _NC = build_nc()
    return _NC


def kernel(x, Wqkv, Wproj, bproj, T, H, W):
    x = np.asarray(x, dtype=np.float32)
    Wqkv = np.asarray(Wqkv, dtype=np.float32)
    Wproj = np.asarray(Wproj, dtype=np.float32)
    bproj = np.asarray(bproj, dtype=np.float32)
    assert x.shape == (B, N, C) and Wqkv.shape == (C, 3 * C)
    assert (int(T), int(H), int(W)) == (GT, GH, GW)

    cos96, sin96 = _cos_sin_96()
    bf = ml_dtypes.bfloat16
    nc = _get_nc()

    in_maps = []
    for core in range(NCORES):
        b, g = divmod(core, HPG)
        heads = [HPG * g + i for i in range(HPG)]
        q_e = [h * HD + 2 * j for h in heads for j in range(24)]
        q_o = [h * HD + 2 * j + 1 for h in heads for j in range(24)]
        q_p = [h * HD + 48 + j for h in heads for j in range(24)]
        wqk_c = np.concatenate(
            [Wqkv[:, q_e], Wqkv[:, q_o], Wqkv[:, q_p],
             Wqkv[:, [C + i for i in q_e]], Wqkv[:, [C + i for i in q_o]],
             Wqkv[:, [C + i for i in q_p]]],
            axis=1,
        )
        wv_c = Wqkv[:, 2 * C + heads[0] * HD: 2 * C + (heads[-1] + 1) * HD]
        # packed wp rows per NORM_MAP chunk layout
        wp_c = np.zeros((128, 3, C), dtype=np.float32)
        for h in range(HPG):
            gr = heads[h] * HD
            for (c, r0, d0, d1) in NORM_MAP[h]:
                wp_c[r0:r0 + (d1 - d0), c, :] = Wproj[gr + d0:gr + d1, :]
        if g == 0:
            wp_c[32, 2, :] = bproj
        in_maps.append({
            "xT": np.ascontiguousarray(x[b].T).astype(bf),
            "wqk": wqk_c.astype(bf),
            "wv": np.ascontiguousarray(wv_c).astype(bf),
            "wpd": wp_c.astype(bf),
            "cosd": cos96.astype(bf),
            "sind": sin96.astype(bf),
            "identd": np.eye(128, dtype=np.float32).astype(bf),
        })

    res = run_bass_kernel_spmd(nc, in_maps, core_ids=list(range(NCORES)))
    global _last_res
    _last_res = res
    out = np.zeros((B, N, C), dtype=np.float32)
    for core in range(NCORES):
        b = core // HPG
        out[b] += res.results[core]["outT"].T
    return out
